# revision 12
# baseline (speedup 1.0000x reference)
"""GCNConv layer (DGL GraphConv norm='both' + self-loop + edge-feature mean)
on 8 Trainium2 NeuronCores — v5.

  out = [(A @ h)*nd + h*scs] @ W + [(A_e @ efeat)*nd^2] @ We   (+bias, zero here)
  h = nfeat * ns,  ns = clip(out_deg,1)^-1/2, nd = clip(in_deg,1)^-1/2,
  scs = inv1/ns,  inv1 = 1/(in_deg+1)

v5 design (all building blocks device-verified):
  - The per-edge [h[src] || efeat] stream, and the h self-loop table, are
    float8e3 (E3M4): halves the dominant HBM traffic. h is quantized BEFORE
    W (W applied per dst block at the tail) so the W-mixing whitens the fp8
    quantization noise: max-rel error ~1.6e-2 vs the 2e-2 gate (quantizing
    hW after W measures 1.9e-2; fp8e4 fails at 2.8e-2).
  - sel stays bf16 (exact one-hot * nd); PE accepts bf16 x fp8e3 matmuls.
  - Launch A has no matmuls: it computes degree scalars and h = nf*ns
    (node-major, per-block scale, E3M4 out). W moved to launch B's tail.
  - Launch B tail is a 3-stage software pipeline (part1 stt/ze frees the
    agg PSUM; part2 transposes+copies; part3 oT matmuls + output copy),
    with stages emitted a few chunk-events apart so PE never head-of-line
    blocks on the DVE/Act chain.
  - Startup DMA order: small sel scalars first, h table + scalar-stream
    remainders interleaved behind the first comb groups.
  - Packed leftover chunks build ONE wide sel [P, R*128] per chunk.

Structure is otherwise v3: edge-parallel, dst-sharded; host does all index
routing (offsets, gathers, permutations), device does all value arithmetic.
"""
import sys
import numpy as np

sys.path.insert(0, "/opt/trn_rl_repo")

P = 128
D = 128
NCORES = 8
N_NODES = 100000
NSH = 12544          # nodes per core (padded: 8*12544 = 100352)
NB = NSH // P        # 98 blocks per core
GB = 11              # chunks per stream DMA group
NBG = 8              # blocks per outT DMA batch
R_MAX = 3            # max regions packed into one leftover chunk


def _host_prep(nfeat, efeat, src, dst):
    E = src.shape[0]
    src = src.astype(np.int64)
    dst = dst.astype(np.int64)

    # ---------------- global degree offsets (index metadata) ----------------
    out_cnt = np.bincount(src, minlength=NCORES * NSH).astype(np.int64)
    in_cnt = np.bincount(dst, minlength=NCORES * NSH).astype(np.int64)
    out_off = np.concatenate([[0], np.cumsum(out_cnt)])
    in_off = np.concatenate([[0], np.cumsum(in_cnt)])

    def offs_pc(off):  # [NCORES*NSH+1] -> starts/ends [NCORES, P, NB]
        starts = off[:-1].reshape(NCORES, NB, P).transpose(0, 2, 1)
        ends = off[1:].reshape(NCORES, NB, P).transpose(0, 2, 1)
        return (np.ascontiguousarray(starts).astype(np.float32),
                np.ascontiguousarray(ends).astype(np.float32))

    srcS, srcE = offs_pc(out_off)
    dstS, dstE = offs_pc(in_off)

    # ---------------- dst-sharded slot layout with block matching ----------
    core = dst // NSH
    block = (dst % NSH) // P
    dstl = (dst % P).astype(np.float32)

    counts = np.zeros((NCORES, NB), dtype=np.int64)
    np.add.at(counts, (core, block), 1)
    ordb = np.argsort(-counts, axis=1, kind="stable")      # [NCORES, NB] block at slot k
    # Visit order: first-fit-decreasing bin packing of the leftover rows,
    # bins interleaved big/small by capacity. Identical on all cores.
    _sc = np.take_along_axis(counts, ordb, axis=1)
    _cap = np.maximum(_sc.max(axis=0), 1)
    _rem = _cap % P
    _ranks = np.argsort(-_rem, kind="stable")
    _bins, _fill = [], []
    for _r in _ranks:
        for _bi in range(len(_bins)):
            if _fill[_bi] + _rem[_r] <= P and len(_bins[_bi]) < R_MAX:
                _bins[_bi].append(_r)
                _fill[_bi] += _rem[_r]
                break
        else:
            _bins.append([_r])
            _fill.append(_rem[_r])
    _bc = np.array([max(_cap[_b]) for _b in _bins])
    _bo = np.argsort(-_bc)
    _inter = []
    for _i in range(len(_bo) // 2 + 1):
        if _i < len(_bo):
            _inter.append(_bo[_i])
        _j = len(_bo) - 1 - _i
        if _j > _i:
            _inter.append(_bo[_j])
    _perm = np.array([_s for _bi in _inter for _s in _bins[_bi]])
    ordb = np.ascontiguousarray(ordb[:, _perm])
    inv_ord = np.empty_like(ordb)
    for c in range(NCORES):
        inv_ord[c, ordb[c]] = np.arange(NB)
    sorted_counts = np.take_along_axis(counts, ordb, axis=1)
    cap = np.maximum(sorted_counts.max(axis=0), 1)          # [NB] per-slot capacity
    full = cap // P
    rem = cap % P

    # ---- build the shared schedule: full chunks + packed leftover chunks ----
    # sched events:
    #   ("full", k, s, cidx, start, stop)       one sel col (code = dst_local)
    #   ("regions", cidx, [(k, win, start)])    ONE wide sel, code = win*128+dstl
    #   ("tail", k)
    sched = []
    fk_start = np.zeros(NB, np.int64)
    pchunk = np.full(NB, -1, np.int64)
    region_lo = np.zeros(NB, np.int64)
    rwin = np.full(NB, -1, np.int64)
    cidx = 0
    pending = []
    fill = 0

    def flush():
        nonlocal cidx, pending, fill
        if not pending:
            return
        regs = []
        for w, (kb, lo) in enumerate(pending):
            pchunk[kb] = cidx
            region_lo[kb] = lo
            rwin[kb] = w
            regs.append((kb, w, bool(full[kb] == 0)))
        sched.append(("regions", cidx, regs))
        cidx += 1
        for (kb, _) in pending:
            sched.append(("tail", kb))
        pending = []
        fill = 0

    for k in range(NB):
        fk_start[k] = cidx
        for s in range(int(full[k])):
            st = (s == 0)
            sp = (s == int(full[k]) - 1) and rem[k] == 0
            sched.append(("full", k, s, cidx, st, sp))
            cidx += 1
        if rem[k] == 0:
            sched.append(("tail", k))
        else:
            if fill + int(rem[k]) > P or len(pending) == R_MAX:
                flush()
            pending.append((k, fill))
            fill += int(rem[k])
    flush()

    NCHP = cidx
    _tail = min(8 * GB, NCHP)
    _head = NCHP - _tail
    _bnds = list(range(0, _head, GB))
    _b = _head
    while _b < NCHP:
        _bnds.append(_b)
        _b += max(GB // 2, 1)
    _bnds.append(NCHP)
    gb_bnds = sorted(set(min(x, NCHP) for x in _bnds))
    SP8 = NCHP * P

    # ---- per-core edge placement ----
    slotk = inv_ord[core, block]
    order = np.lexsort((slotk, core))
    core_s = core[order]
    slotk_s = slotk[order]
    dstl_s = dstl[order]

    e_src = [None] * NCORES
    e_dst = [None] * NCORES
    e_eidx = [None] * NCORES
    e_abs = [None] * NCORES
    e_col = [None] * NCORES
    e_row = [None] * NCORES
    dst_colsM = np.full((NCORES, P, NCHP), -1.0, dtype=np.float32)
    core_starts = np.concatenate([[0], np.cumsum(np.bincount(core_s, minlength=NCORES))])
    for c in range(NCORES):
        lo, hi = core_starts[c], core_starts[c + 1]
        ks = slotk_s[lo:hi]
        cnts = sorted_counts[c]
        within = np.arange(hi - lo) - np.repeat(
            np.concatenate([[0], np.cumsum(cnts)])[:-1], cnts)
        in_full = within < full[ks] * P
        row = np.where(in_full, within % P, region_lo[ks] + (within - full[ks] * P))
        chunk = np.where(in_full, fk_start[ks] + within // P, pchunk[ks])
        code = np.where(in_full, dstl_s[lo:hi], rwin[ks] * P + dstl_s[lo:hi])
        e_src[c] = src[order[lo:hi]]
        e_dst[c] = dst[order[lo:hi]]
        e_eidx[c] = order[lo:hi]
        e_abs[c] = chunk * P + row
        e_col[c] = chunk
        e_row[c] = row
        dst_colsM[c, row, chunk] = code

    iota = np.tile(np.arange(R_MAX * P, dtype=np.int16), (P, 1))

    tail_order = [ev[1] for ev in sched if ev[0] == "tail"]
    return dict(
        sched=sched, NCHP=NCHP, SP8=SP8, tail_order=tail_order,
        ordb=ordb, srcS=srcS, srcE=srcE, dstS=dstS, dstE=dstE,
        e_src=e_src, e_dst=e_dst, e_eidx=e_eidx,
        e_abs=e_abs, e_col=e_col, e_row=e_row, gb_bnds=gb_bnds,
        dst_colsM=dst_colsM, iota=iota,
        in_cnt=in_cnt,
    )


def _build_launch_a(meta):
    import concourse.mybir as mybir
    from concourse import bacc
    from concourse.tile import TileContext

    F32, BF16 = mybir.dt.float32, mybir.dt.bfloat16
    E3 = mybir.dt.float8e3
    AF = mybir.ActivationFunctionType

    nc = bacc.Bacc("TRN2", target_bir_lowering=False, debug=False, num_devices=NCORES)
    nf_nm = nc.dram_tensor("nf_nm", [P, NB * D], BF16, kind="ExternalInput")
    offs = nc.dram_tensor("offs", [P, 4 * NB], F32, kind="ExternalInput")
    h_out = nc.dram_tensor("h_out", [P, NB * D], E3, kind="ExternalOutput")
    nd_out = nc.dram_tensor("nd_out", [P, NB], F32, kind="ExternalOutput")
    scs_out = nc.dram_tensor("scs_out", [P, NB], F32, kind="ExternalOutput")

    NSEG = 7
    SEGB = NB // NSEG
    with TileContext(nc) as tc:
        with tc.tile_pool(name="res", bufs=1) as res, \
             tc.tile_pool(name="hwp", bufs=7) as hwp:
            nf_t = res.tile([P, NB, D], BF16)
            offs_t = res.tile([P, 4, NB], F32)
            warm = res.tile([P, 1], F32)
            warm2 = res.tile([P, 1], F32)
            nc.gpsimd.memset(warm[:], 0.0)
            nc.scalar.activation(out=warm2[:], in_=warm[:], func=AF.Sqrt)
            nc.sync.dma_start(out=offs_t[:].rearrange("p a b -> p (a b)"), in_=offs[:])
            for k in range(NSEG):
                nc.sync.dma_start(
                    out=nf_t[:, k * SEGB:(k + 1) * SEGB, :].rearrange("p a b -> p (a b)"),
                    in_=nf_nm[:, k * SEGB * D:(k + 1) * SEGB * D])

            odeg = res.tile([P, NB], F32)
            om = res.tile([P, NB], F32)
            orc = res.tile([P, NB], F32)
            ns_t = res.tile([P, NB], F32)
            ideg = res.tile([P, NB], F32)
            im = res.tile([P, NB], F32)
            nd_t = res.tile([P, NB], F32)
            i1 = res.tile([P, NB], F32)
            inv1 = res.tile([P, NB], F32)
            rns = res.tile([P, NB], F32)
            scs_t = res.tile([P, NB], F32)

            TT, TS = mybir.AluOpType, mybir.AluOpType
            nc.vector.tensor_tensor(out=odeg[:], in0=offs_t[:, 1, :], in1=offs_t[:, 0, :], op=TT.subtract)
            nc.vector.tensor_scalar(out=om[:], in0=odeg[:], scalar1=1.0,
                                    scalar2=None, op0=TS.max)
            nc.vector.reciprocal(out=orc[:], in_=om[:])
            nc.scalar.activation(out=ns_t[:], in_=orc[:], func=AF.Sqrt)
            nc.vector.tensor_tensor(out=ideg[:], in0=offs_t[:, 3, :], in1=offs_t[:, 2, :], op=TT.subtract)
            nc.vector.tensor_scalar(out=im[:], in0=ideg[:], scalar1=1.0,
                                    scalar2=None, op0=TS.max)
            irc = res.tile([P, NB], F32)
            nc.vector.reciprocal(out=irc[:], in_=im[:])
            nc.scalar.activation(out=nd_t[:], in_=irc[:], func=AF.Sqrt)
            nc.vector.tensor_scalar(out=i1[:], in0=ideg[:], scalar1=1.0,
                                    scalar2=None, op0=TS.add)
            nc.vector.reciprocal(out=inv1[:], in_=i1[:])
            nc.scalar.activation(out=rns[:], in_=om[:], func=AF.Sqrt)
            nc.vector.tensor_tensor(out=scs_t[:], in0=inv1[:], in1=rns[:], op=TT.mult)
            nc.sync.dma_start(out=nd_out[:], in_=nd_t[:])
            nc.sync.dma_start(out=scs_out[:], in_=scs_t[:])

            h_t = None
            for j in range(NB):
                if j % SEGB == 0:
                    h_t = hwp.tile([P, SEGB, D], E3, tag="hseg")
                # DVE is cheaper per op here; give it ~2/3 of the blocks
                if j % 4 == 3:
                    nc.scalar.activation(out=h_t[:, j % SEGB, :], in_=nf_t[:, j, :],
                                         func=AF.Copy, scale=ns_t[:, j:j + 1])
                else:
                    nc.vector.tensor_scalar(out=h_t[:, j % SEGB, :], in0=nf_t[:, j, :],
                                            scalar1=ns_t[:, j:j + 1], scalar2=None,
                                            op0=TS.mult)
                if j % SEGB == SEGB - 1 or j == NB - 1:
                    k0 = (j // SEGB) * SEGB
                    nc.sync.dma_start(
                        out=h_out[:, k0 * D:(j + 1) * D],
                        in_=h_t[:, 0:(j - k0 + 1), :].rearrange("p a b -> p (a b)"))
    nc.compile()
    return nc


def _build_launch_b(meta):
    import concourse.mybir as mybir
    from concourse import bacc
    from concourse.tile import TileContext

    F32, BF16, I16 = mybir.dt.float32, mybir.dt.bfloat16, mybir.dt.int16
    E3 = mybir.dt.float8e3
    AF = mybir.ActivationFunctionType
    TS = mybir.AluOpType
    NCHP = meta["NCHP"]
    sched = meta["sched"]

    nc = bacc.Bacc("TRN2", target_bir_lowering=False, debug=False, num_devices=NCORES)
    gb_bnds = meta["gb_bnds"]
    comb = nc.dram_tensor("comb", [P, NCHP * 2 * D], E3, kind="ExternalInput")
    dstc = nc.dram_tensor("dstc", [P, NCHP], F32, kind="ExternalInput")
    ndE = nc.dram_tensor("ndE", [P, NCHP], F32, kind="ExternalInput")
    smalls = nc.dram_tensor("smalls", [P, 2 * NB + 2 * (min(8 * GB, NCHP))], F32, kind="ExternalInput")
    iota = nc.dram_tensor("iota", [P, R_MAX * P], I16, kind="ExternalInput")
    w_in = nc.dram_tensor("w_in", [D, D], F32, kind="ExternalInput")
    we_in = nc.dram_tensor("we_in", [D, D], F32, kind="ExternalInput")
    identity = nc.dram_tensor("identity", [P, P], BF16, kind="ExternalInput")
    hb = nc.dram_tensor("hb", [P, NB * D], E3, kind="ExternalInput")
    outT = nc.dram_tensor("outT", [D, NSH], BF16, kind="ExternalOutput")

    # scalar-stream split: first SCEARLY chunk cols ride in the packed
    # "smalls" startup DMA, rest loads behind the early comb groups
    SCEARLY = min(8 * GB, NCHP)
    HSEG = 7
    HSEGB = NB // HSEG

    with TileContext(nc) as tc:
        with tc.tile_pool(name="res", bufs=1) as res, \
             tc.tile_pool(name="cp", bufs=8) as cpp, \
             tc.tile_pool(name="selp", bufs=32) as selp, \
             tc.tile_pool(name="wselp", bufs=4) as wselp, \
             tc.tile_pool(name="hp", bufs=10) as hp, \
             tc.tile_pool(name="agg_ps", bufs=4, space="PSUM") as aggp, \
             tc.tile_pool(name="tr_ps", bufs=2, space="PSUM") as trp, \
             tc.tile_pool(name="out_ps", bufs=2, space="PSUM") as outp, \
             tc.tile_pool(name="ob", bufs=2) as obp:
            iota_t = res.tile([P, R_MAX * P], I16)
            dst_t = res.tile([P, NCHP], F32)
            ndE_t = res.tile([P, NCHP], F32)
            w_t = res.tile([D, D], BF16)
            we_t = res.tile([D, D], BF16)
            id_t = res.tile([P, P], BF16)
            hb_t = res.tile([P, NB, D], E3)
            sm_t = res.tile([P, 2 * NB + 2 * SCEARLY], F32)
            warm = res.tile([P, 1], F32)
            warm2 = res.tile([P, 1], F32)
            nc.gpsimd.memset(warm[:], 0.0)
            nc.scalar.activation(out=warm2[:], in_=warm[:], func=AF.Copy)
            # ---- startup: minimum DMA count before the first comb group ----
            nc.sync.dma_start(out=sm_t[:], in_=smalls[:])
            nc.sync.dma_start(out=iota_t[:], in_=iota[:])
            nc.gpsimd.dma_start(out=w_t[:], in_=w_in[:])     # f32 -> bf16 cast
            nc.gpsimd.dma_start(out=we_t[:], in_=we_in[:])

            state = {"cb": None, "sel_n": 0, "t": 0}
            aggs = {}
            # deferred tail pipeline
            tq1 = []   # (j, zadd, ze) awaiting transposes
            tq2 = []   # (pair, age) awaiting oT matmuls
            pairs = {"q": None, "oT": None, "ob": None, "npair": 0}

            import bisect as _bisect

            def load_group(cidx):
                g = _bisect.bisect_right(gb_bnds, cidx) - 1
                lo, hi = gb_bnds[g], gb_bnds[g + 1]
                o = cidx - lo
                if o == 0:
                    # interleave deferred bulk loads behind the early groups
                    if g == 1:
                        nc.sync.dma_start(out=id_t[:], in_=identity[:])
                    if g % 2 == 0 and 2 <= g <= 2 * HSEG:
                        s = g // 2 - 1
                        j0, j1 = s * HSEGB, NB if s == HSEG - 1 else (s + 1) * HSEGB
                        if j0 < j1:
                            nc.scalar.dma_start(
                                out=hb_t[:, j0:j1, :].rearrange("p a b -> p (a b)"),
                                in_=hb[:, j0 * D:j1 * D])
                    if g == 3 and SCEARLY < NCHP:
                        nc.sync.dma_start(out=dst_t[:, SCEARLY:], in_=dstc[:, SCEARLY:])
                    if g == 5 and SCEARLY < NCHP:
                        nc.sync.dma_start(out=ndE_t[:, SCEARLY:], in_=ndE[:, SCEARLY:])
                    cb = cpp.tile([P, GB, 2 * D], E3, tag="comb")
                    geng = (nc.sync, nc.scalar)[g % 2]
                    geng.dma_start(
                        out=cb[:, 0:hi - lo, :].rearrange("p g f -> p (g f)"),
                        in_=comb[:, lo * 2 * D:hi * 2 * D])
                    state["cb"] = cb
                return state["cb"], o

            def build_sel(cidx, width):
                if width == P:
                    sel = selp.tile([P, P], BF16, tag="sel")
                    seng = nc.gpsimd if (state["sel_n"] * 2) % 5 < 2 else nc.vector
                else:
                    sel = wselp.tile([P, R_MAX * P], BF16, tag="wsel")
                    seng = nc.vector
                state["sel_n"] += 1
                if cidx < SCEARLY:
                    s1 = sm_t[:, 2 * NB + cidx:2 * NB + cidx + 1]
                    s2 = sm_t[:, 2 * NB + SCEARLY + cidx:2 * NB + SCEARLY + cidx + 1]
                else:
                    s1 = dst_t[:, cidx:cidx + 1]
                    s2 = ndE_t[:, cidx:cidx + 1]
                seng.tensor_scalar(
                    out=sel[:, 0:width], in0=iota_t[:, 0:width],
                    scalar1=s1, scalar2=s2, op0=TS.is_equal, op1=TS.mult)
                return sel

            def tail_part1(j):
                # frees the agg PSUM bank; no PE involvement
                agg = aggs.pop(j)
                zadd = hp.tile([P, D], BF16, tag="zadd")
                ze = hp.tile([P, D], BF16, tag="ze")
                nc.vector.scalar_tensor_tensor(
                    out=zadd[:], in0=hb_t[:, j, :], scalar=sm_t[:, NB + j:NB + j + 1],
                    in1=agg[:, 0:D], op0=TS.mult, op1=TS.add)
                nc.scalar.activation(out=ze[:], in_=agg[:, D:2 * D],
                                     func=AF.Copy, scale=sm_t[:, j:j + 1])
                tq1.append((j, zadd, ze))

            def tail_part2():
                # one PAIR: 4 PE transposes into one PSUM tile + 2 wide copies
                (ja, za, ea) = tq1.pop(0)
                (jb, zb, eb) = tq1.pop(0)
                tr = trp.tile([P, 4, D], BF16, tag="tr")
                nc.tensor.transpose(out=tr[:, 0, :], in_=za[:], identity=id_t[:])
                nc.tensor.transpose(out=tr[:, 1, :], in_=zb[:], identity=id_t[:])
                nc.tensor.transpose(out=tr[:, 2, :], in_=ea[:], identity=id_t[:])
                nc.tensor.transpose(out=tr[:, 3, :], in_=eb[:], identity=id_t[:])
                t_sb = hp.tile([P, 4, D], BF16, tag="tsb")
                nc.scalar.activation(
                    out=t_sb[:].rearrange("p a b -> p (a b)"),
                    in_=tr[:].rearrange("p a b -> p (a b)"), func=AF.Copy)
                tq2.append([t_sb, 0])

            def tail_part3():
                t_sb, _ = tq2.pop(0)
                npair = pairs["npair"]
                pairs["npair"] = npair + 1
                q = npair % 2            # pair slot within the oT tile
                if q == 0:
                    pairs["oT"] = outp.tile([P, 4, D], F32, tag="oT", name="oT")
                if npair % (NBG // 2) == 0:
                    pairs["ob"] = obp.tile([P, NBG, D], BF16, tag="ob", name="ob")
                oT, ob_t = pairs["oT"], pairs["ob"]
                for h in range(2):
                    nc.tensor.matmul(out=oT[:, 2 * q + h, :], lhsT=w_t[:],
                                     rhs=t_sb[:, h, :], start=True, stop=False)
                    nc.tensor.matmul(out=oT[:, 2 * q + h, :], lhsT=we_t[:],
                                     rhs=t_sb[:, 2 + h, :], start=False, stop=True)
                nblk = 2 * (npair + 1)
                go = (npair // 2) % 2    # oT slot within the ob tile
                if q == 1:
                    nc.scalar.activation(
                        out=ob_t[:, 4 * go:4 * go + 4, :].rearrange("p a b -> p (a b)"),
                        in_=oT[:].rearrange("p a b -> p (a b)"), func=AF.Copy)
                elif nblk == NB:
                    nc.scalar.activation(
                        out=ob_t[:, 4 * go:4 * go + 2, :].rearrange("p a b -> p (a b)"),
                        in_=oT[:, 0:2, :].rearrange("p a b -> p (a b)"), func=AF.Copy)
                if nblk % NBG == 0 or nblk == NB:
                    g2 = (nblk - 1) // NBG
                    w_blocks = nblk - g2 * NBG
                    nc.scalar.dma_start(
                        out=outT[:, g2 * NBG * P:g2 * NBG * P + w_blocks * P],
                        in_=ob_t[:, 0:w_blocks, :].rearrange("p a b -> p (a b)"))

            def pump(endgame=False):
                # advance the deferred tail pipeline
                for item in tq2:
                    item[1] += 1
                age = 2 if endgame else 6
                if tq2 and tq2[0][1] >= age:
                    tail_part3()
                if len(tq1) >= (2 if endgame else 3):
                    tail_part2()

            n_ev = len(sched)
            for ev_i, ev in enumerate(sched):
                if ev[0] == "full":
                    _, j, s, cidx, st, sp = ev
                    cb, o = load_group(cidx)
                    if st:
                        aggs[j] = aggp.tile([P, 2 * D], F32, tag="agg", name="agg")
                    sel = build_sel(cidx, P)
                    nc.tensor.matmul(out=aggs[j][:], lhsT=sel[:, 0:P], rhs=cb[:, o, :],
                                     start=st, stop=sp)
                    pump(endgame=ev_i > n_ev - 120)
                elif ev[0] == "regions":
                    _, cidx, regs = ev
                    cb, o = load_group(cidx)
                    sel = build_sel(cidx, len(regs) * P)
                    for (j, win, st) in regs:
                        if st:
                            aggs[j] = aggp.tile([P, 2 * D], F32, tag="agg", name="agg")
                        nc.tensor.matmul(out=aggs[j][:],
                                         lhsT=sel[:, win * P:(win + 1) * P],
                                         rhs=cb[:, o, :], start=st, stop=True)
                    pump()
                else:
                    tail_part1(ev[1])
            # drain the tail pipeline
            while tq1 or tq2:
                if tq2:
                    tail_part3()
                elif len(tq1) >= 2:
                    tail_part2()
    nc.compile()
    return nc


def kernel(nfeat, efeat, src, dst, W, b, We, be):
    import ml_dtypes
    from concourse import bass_utils

    nfeat = np.asarray(nfeat, dtype=np.float32)
    efeat = np.asarray(efeat, dtype=np.float32)
    W = np.asarray(W, dtype=np.float32)
    b = np.asarray(b, dtype=np.float32)
    We = np.asarray(We, dtype=np.float32)
    be = np.asarray(be, dtype=np.float32)
    src = np.asarray(src)
    dst = np.asarray(dst)

    meta = _host_prep(nfeat, efeat, src, dst)
    BF = ml_dtypes.bfloat16
    E3 = ml_dtypes.float8_e3m4

    nfeat_pad = np.concatenate(
        [nfeat, np.zeros((NCORES * NSH - N_NODES, D), np.float32)], axis=0)

    # ---------- launch A ----------
    ncA = _build_launch_a(meta)
    in_maps_a = []
    for c in range(NCORES):
        nf_nm = np.ascontiguousarray(
            nfeat_pad[c * NSH:(c + 1) * NSH].reshape(NB, P, D).transpose(1, 0, 2)
        ).reshape(P, NB * D).astype(BF)
        in_maps_a.append({
            "nf_nm": nf_nm,
            "offs": np.ascontiguousarray(np.stack(
                [meta["srcS"][c], meta["srcE"][c], meta["dstS"][c], meta["dstE"][c]],
                axis=1)).reshape(P, 4 * NB),
        })
    resA = bass_utils.run_bass_kernel_spmd(ncA, in_maps_a, core_ids=list(range(NCORES)))

    # ---------- host glue: gather h[src] (e3m4 bytes), efeat, route nd ----------
    h_parts, nd_parts = [], []
    for c in range(NCORES):
        h = resA.results[c]["h_out"].reshape(P, NB, D)
        h_parts.append(np.ascontiguousarray(h.transpose(1, 0, 2)).reshape(NSH, D))
        nd_parts.append(resA.results[c]["nd_out"].T.reshape(-1))   # node n=j*128+p
    h_full = np.concatenate(h_parts, axis=0)                       # [NCORES*NSH, D] e3m4
    nd_tab = np.concatenate(nd_parts, axis=0)                      # [NCORES*NSH] f32

    SP8, NCHP = meta["SP8"], meta["NCHP"]
    SCEARLY = min(8 * GB, NCHP)
    efeat_e3 = efeat.astype(E3)
    comb = np.zeros((NCORES, SP8, 2 * D), dtype=E3)
    ndE_cols = np.zeros((NCORES, P, NCHP), dtype=np.float32)
    for c in range(NCORES):
        ab = meta["e_abs"][c]
        comb[c, ab, 0:D] = h_full[meta["e_src"][c]]
        comb[c, ab, D:2 * D] = efeat_e3[meta["e_eidx"][c]]
        ndE_cols[c, meta["e_row"][c], meta["e_col"][c]] = nd_tab[meta["e_dst"][c]]
    comb_blk = np.ascontiguousarray(
        comb.reshape(NCORES, NCHP, P, 2 * D).transpose(0, 2, 1, 3)
        .reshape(NCORES, P, NCHP * 2 * D))

    identity = np.eye(P).astype(BF)

    # ---------- launch B ----------
    ncB = _build_launch_b(meta)
    in_maps_b = []
    for c in range(NCORES):
        ordc = meta["ordb"][c]
        hbv = resA.results[c]["h_out"].reshape(P, NB, D)[:, ordc, :]
        ndB = resA.results[c]["nd_out"][:, ordc]
        scs = resA.results[c]["scs_out"][:, ordc]
        in_maps_b.append({
            "comb": comb_blk[c],
            "dstc": meta["dst_colsM"][c],
            "ndE": ndE_cols[c],
            "iota": meta["iota"],
            "w_in": W,
            "we_in": We,
            "identity": identity,
            "hb": np.ascontiguousarray(hbv).reshape(P, NB * D),
            "smalls": np.ascontiguousarray(np.concatenate(
                [ndB, scs, meta["dst_colsM"][c][:, 0:SCEARLY],
                 ndE_cols[c][:, 0:SCEARLY]], axis=1)),
        })
    resB = bass_utils.run_bass_kernel_spmd(ncB, in_maps_b, core_ids=list(range(NCORES)))

    tail_order = np.array(meta["tail_order"], dtype=np.int64)
    out_parts = []
    for c in range(NCORES):
        oT = resB.results[c]["outT"].astype(np.float32)    # [D, NSH] tail-emission order
        o = oT.T.reshape(NB, P, D)                         # [t, p, D]
        natural_of_t = meta["ordb"][c][tail_order]         # block id per t
        inv = np.empty(NB, dtype=np.int64)
        inv[natural_of_t] = np.arange(NB)
        out_parts.append(o[inv].reshape(NSH, D))
    out = np.concatenate(out_parts, axis=0)[:N_NODES]

    if np.abs(b).max() > 0 or np.abs(be).max() > 0:
        in_deg = meta["in_cnt"][:N_NODES].astype(np.float32)
        out = out + b[None, :] * (1.0 + 1.0 / (in_deg[:, None] + 1.0)) \
                  + be[None, :] * (in_deg[:, None] > 0)
    return np.ascontiguousarray(out.astype(np.float32))
